# revision 55
# baseline (speedup 1.0000x reference)
"""Trainium2 Bass kernel for MoE feed-forward (nn_MoEFeedForward_12292196401617).

Reference computation (per batch b of 32, N=1024 tokens, DIM=1024):
    h      = gelu_erf(x @ fc1_w.T + fc1_b)                  # [B,N,HID=4096]
    shared = h @ fc2_w.T + fc2_b                            # [B,N,768]
    expert = h @ experts_w[idx[b]].T + experts_b[idx[b]]    # [B,N,256]
    out    = concat([shared, expert], -1)                   # [B,N,1024]

Strategy: data-parallel over batch across 8 NeuronCores (4 batches/core).
The expert gather is resolved on the host (indices are per-batch), so the
device program is pure dense matmul, feature-on-partitions / tokens-on-free,
host-packed so every DMA row is a multi-KB contiguous DRAM run.

Precision: the kernel is PE-streaming-bound at the fp16 rate (1 row/cycle,
~874us/core), so the last 2 of 8 fc1 contraction chunks AND the last 2 of
32 fc2 chunks each run as a single fp8-e4m3 DoubleRow matmul (2 rows/
cycle, rhs free=1024): ~209ns saved per chain (DR matmul measured ~223ns
vs 432ns for 2 fp16 MMs, its 136ns LDWEIGHTS hides under the previous MM).
Scale matching lets each fp8 MM accumulate into the same PSUM group as the
fp16 MMs: operands are pre-scaled by powers of 2 in BOTH dtypes (exact in
fp16) — x*16 and fc1_w*512 (all products carry 8192, gelu eviction scale
1/8192), fc2_w/experts_w*1024 with h unscaled (products carry 1024,
identity eviction scale 1/1024; h chunks 30,31 are evicted by the gelu
straight to fp8). Quantization error scales as sqrt(fraction quantized):
full-fp8 fails the 2e-2 gate at ~5e-2, this split measures 1.8880e-2 —
bit-stable across runs and digit-identical to the numpy simulation, so the
margin is deterministic, not statistical. fp16-only measured 3.8e-4.
Output is stored fp16 and upcast on the host. Measured ~839us (baseline
fp16 kernel: 907us): ~824us matmul span at ~98% PE busy + ~8us NEFF
preamble + ~3.6us warmup bridge + ~4us ramp holes + ~4us epilogue.
"""

import sys

sys.path.insert(0, "/opt/trn_rl_repo")

import numpy as np
import ml_dtypes

B, N, DIM = 32, 1024, 1024
HID = 4096
PART = 256
OUT = 1024
SHARED = OUT - PART  # 768
E = 16

NCORES = 8
BPC = B // NCORES        # batches per core = 4
TOK = BPC * N            # tokens per core  = 4096
TT = 512                 # token tile
NTILES = TOK // TT       # 8  (2 tiles per batch)
P = 128
KC = DIM // P            # 8  fc1 contraction chunks
KF = 6                   # fc1 chunks 0..5 in fp16; 6,7 in one fp8 DoubleRow
HC = HID // P            # 32 hidden chunks
OC = OUT // P            # 8  output chunks (6 shared + 2 expert)
SC = SHARED // P         # 6
W1Q = 16                 # w1 column-slice groups (HC/W1Q = 2 hid chunks each)
W18G = 4                 # fp8 w1 DMA split along hid (2KB rows; finer
                         # splits lower packet-rate-limited early DMA BW)
HF = 30                  # fc2 chunks 0..29 in fp16; 30,31 in one fp8 DoubleRow
W2G = 6                  # fp16 w2 hid chunks packed per DMA (5 groups of 6)
XS = 16.0                # x pre-scale  (both dtypes)
WS = 512.0               # w1 pre-scale (both dtypes)
W2S = 1024.0             # w2/experts pre-scale (both dtypes)

_CACHE: dict = {}


def _build_program():
    import concourse.tile as tile
    from concourse import bacc, mybir

    fp16 = mybir.dt.float16
    fp8 = mybir.dt.float8e4
    f32 = mybir.dt.float32
    GELU = mybir.ActivationFunctionType.Gelu
    IDENT = mybir.ActivationFunctionType.Identity
    DR = mybir.MatmulPerfMode.DoubleRow

    HQ = HID // W1Q          # 256 hid cols per w1 fp16 slice group
    H8 = HID // W18G         # 1024 hid cols per w1 fp8 DMA group
    nc = bacc.Bacc()
    # packed layouts: [.., P, ..] second-to-last dim is the SBUF partition,
    # the trailing dims are one contiguous row per partition.
    xF_d = nc.declare_dram_parameter("xF", [NTILES, P, KF * TT], fp16, isOutput=False)
    x8_d = nc.declare_dram_parameter("x8", [NTILES, P, 2 * TT], fp8, isOutput=False)
    w1F_d = nc.declare_dram_parameter("w1F", [W1Q, P, KF * HQ], fp16, isOutput=False)
    w18_d = nc.declare_dram_parameter("w18", [W18G, P, 2 * H8], fp8, isOutput=False)
    b1T_d = nc.declare_dram_parameter("b1T", [P, HC], f32, isOutput=False)
    w2P_d = nc.declare_dram_parameter("w2P", [HF // W2G, P, W2G * SHARED], fp16, isOutput=False)
    w28_d = nc.declare_dram_parameter("w28", [P, 2 * SHARED], fp8, isOutput=False)
    weP_d = nc.declare_dram_parameter("weP", [BPC, P, HF * PART], fp16, isOutput=False)
    we8_d = nc.declare_dram_parameter("we8", [BPC, P, 2 * PART], fp8, isOutput=False)
    b2T_d = nc.declare_dram_parameter("b2T", [P, BPC * OC], f32, isOutput=False)
    outT_d = nc.declare_dram_parameter("outT", [OUT, TOK], fp16, isOutput=True)

    with tile.TileContext(nc) as tc:
        with (
            tc.tile_pool(name="wsb", bufs=1) as wsb,      # resident weights
            tc.tile_pool(name="wesb", bufs=1) as wesb,    # expert weights (per batch)
            tc.tile_pool(name="bsb", bufs=1) as bsb,      # biases
            tc.tile_pool(name="xsb", bufs=2) as xsb,      # x fp16 tiles, double buffered
            tc.tile_pool(name="x8sb", bufs=2) as x8sb,    # x fp8 tiles
            tc.tile_pool(name="hsb", bufs=1) as hsb,      # gelu output chunks
            tc.tile_pool(name="osb", bufs=4) as osb,      # out staging
            tc.tile_pool(name="hps", bufs=4, space="PSUM") as hps,
            tc.tile_pool(name="ops", bufs=4, space="PSUM") as ops,
        ):
            # ---- load order matters: the first fc1 chain needs x tile 0,
            # w1F group 0, x8 tile 0 and w18 group 0 within ~1.5us of compute
            # start; w2 / expert weights aren't read until the PE is ~50us
            # in, so they load behind the critical path.
            # b1/b2 are tiny (16KB each) but must NOT ride the gpsimd
            # software-DMA queue: its start time is erratic (10.6-15.5us
            # observed) while the first gelu eviction reads b1 at ~16us —
            # a straggle would stall every fc1 eviction. They ride the
            # sync ring just behind the first-chain prefix instead.
            b1_t = bsb.tile([P, HC], f32, tag="b1")
            b2_t = bsb.tile([P, BPC * OC], f32, tag="b2")

            # PE warmup: dummy matmuls on a memset scratch tile keep the PE
            # busy from the preamble until the first x/w1 bytes land, so the
            # HAM clock gate reaches 8/8 before real work starts. The
            # results are never read.
            scr = bsb.tile([P, TT], fp16, tag="scr")
            nc.vector.memset(scr[:], 0.0)
            # Engine queues are barrier-held until ~8us (NEFF preamble), so
            # the bridge covers ~8.0us -> first-chain data arrival (~10.5us
            # at the ~250-270GB/s early DMA rate; the qSync ring start
            # jitters run-to-run by ~1.5us). 18 dummies = ~3.8us of
            # continuous PE activity: reliably spans one full 3.41us HAM
            # SHORT window so the clock is at 8/8 before the first real
            # chain (15 dummies = 3.2us measured a cold ramp: every early
            # matmul at 427ns instead of 216ns). The qAct ring initializes
            # ~1.4-3.4us later than qSync, so critical loads must NOT ride
            # it (measured: moving w1F[0]/x8/w18[0] there cost 1.6-4us).
            for _ in range(18):
                wp = hps.tile([P, TT], f32, tag="hps", name="warm")
                nc.tensor.matmul(
                    wp[:, 0:256], scr[:, 0:P], scr[:, 0:256], start=True, stop=True
                )

            def load_x(ti):
                # three DMAs per token tile: the fc1 chains' kc=0..2 matmuls
                # only depend on the first fp16 half, so the PE can start
                # before the full tile lands.
                t = xsb.tile([P, KF * TT], fp16, tag="xt", name="xt")
                half = KF * TT // 2
                nc.sync.dma_start(t[:, 0:half], xF_d[ti, :, 0:half])
                t8 = x8sb.tile([P, 2, TT], fp8, tag="x8", name="x8")
                nc.sync.dma_start(t8[:], x8_d[ti])
                nc.sync.dma_start(t[:, half:], xF_d[ti, :, half:])
                return t, t8

            def load_we(b):
                # two DMAs per batch: fp16 [P, HF*PART] (15KB rows) + the
                # fp8 DoubleRow tail [P, 2, PART]
                t = wesb.tile([P, HF * PART], fp16, tag="we", name="we")
                nc.sync.dma_start(t[:], weP_d[b])
                t8 = wesb.tile([P, 2, PART], fp8, tag="we8", name="we8")
                nc.sync.dma_start(t8[:], we8_d[b])
                return t, t8

            # Tile 0 is loaded inline, split fine-grained and ordered by
            # when each byte is first needed: chain 0's kc0,1 matmuls are
            # runnable after a 450KB prefix (xF kc0-1 + w1F[0] kc0-2 half),
            # so even when the DMA ring starts late the PE gets real work
            # in a trickle instead of one long idle that re-throttles HAM.
            xt0 = xsb.tile([P, KF * TT], fp16, tag="xt", name="xt")
            nc.sync.dma_start(xt0[:, 0:2 * TT], xF_d[0, :, 0:2 * TT])
            w1_t = []
            w18_t = wsb.tile([P, 2, HID], fp8, tag="w18", name="w18")
            t = wsb.tile([P, KF * HQ], fp16, tag="w1_0", name="w1_0")
            nc.sync.dma_start(t[:], w1F_d[0])
            w1_t.append(t)
            # w1F[1] rides BEFORE the bulk x bytes: the ti==0 head issues
            # kc-major across four open chains, so one early weight group
            # unlocks 2x the runnable matmuls per arriving x byte.
            t = wsb.tile([P, KF * HQ], fp16, tag="w1_1", name="w1_1")
            nc.sync.dma_start(t[:], w1F_d[1])
            w1_t.append(t)
            nc.sync.dma_start(xt0[:, 2 * TT:3 * TT], xF_d[0, :, 2 * TT:3 * TT])
            nc.sync.dma_start(xt0[:, 3 * TT:], xF_d[0, :, 3 * TT:])
            # w1F[2] also precedes the DR operands: head chains 4,5 run
            # their fp16 matmuls while x8/w18 are still in flight.
            t = wsb.tile([P, KF * HQ], fp16, tag="w1_2", name="w1_2")
            nc.sync.dma_start(t[:], w1F_d[2])
            w1_t.append(t)
            t = wsb.tile([P, KF * HQ], fp16, tag="w1_3", name="w1_3")
            nc.sync.dma_start(t[:], w1F_d[3])
            w1_t.append(t)
            x8t0 = x8sb.tile([P, 2, TT], fp8, tag="x8", name="x8")
            nc.sync.dma_start(x8t0[:], x8_d[0])
            nc.sync.dma_start(w18_t[:, :, 0:H8], w18_d[0])
            nc.sync.dma_start(b1_t[:], b1T_d[:, :])
            nc.sync.dma_start(b2_t[:], b2T_d[:, :])
            x_pend = (xt0, x8t0)

            # remaining w1 fp16: W1Q slice-group tiles [P, KF*HQ], one DMA
            # each, earliest-needed first (chain hc uses group
            # hc // (HC//W1Q)), interleaved with the fp8 tail weights.
            for q in range(4, W1Q):
                t = wsb.tile([P, KF * HQ], fp16, tag=f"w1_{q}", name=f"w1_{q}")
                nc.sync.dma_start(t[:], w1F_d[q])
                w1_t.append(t)
                if 0 < q - 3 < W18G:
                    nc.sync.dma_start(
                        w18_t[:, :, (q - 3) * H8:(q - 2) * H8], w18_d[q - 3]
                    )

            # w2: HF//W2G group tiles [P, W2G*SHARED], one DMA each (9KB
            # rows), plus the fp8 DoubleRow tail [P, 2, SHARED]
            w2_t = []
            for g in range(HF // W2G):
                t = wsb.tile([P, W2G * SHARED], fp16, tag=f"w2_{g}", name=f"w2_{g}")
                nc.sync.dma_start(t[:], w2P_d[g])
                w2_t.append(t)
            w28_t = wsb.tile([P, 2, SHARED], fp8, tag="w28", name="w28")
            nc.sync.dma_start(w28_t[:], w28_d[:, :])

            we_cur, we8_cur = load_we(0)

            HPG = HC // W1Q  # hid chunks per w1 slice group = 2
            for ti in range(NTILES):
                b = ti // (NTILES // BPC)
                t0 = ti * TT
                if ti % (NTILES // BPC) == 0 and ti > 0:
                    we_cur, we8_cur = load_we(b)

                x_t, x8_t = x_pend
                if ti + 1 < NTILES:
                    x_pend = load_x(ti + 1)

                # fc1 + erf-gelu: h^T[hid, tok] per 128-row chunk.
                # kc 0..5 are fp16 matmuls; kc 6,7 are one fp8 DoubleRow
                # matmul (slot s = chunk 6+s) accumulating into the same
                # PSUM tile (operand pre-scales make all products 8192*x*w).
                # Chunks hc<HF evict to fp16; hc 30,31 evict straight to the
                # fp8 tile feeding fc2's own DoubleRow tail.
                h_t = []
                h8_t = hsb.tile([P, 2, TT], fp8, tag="h8")
                if ti == 0:
                    # Ramp head: the first four chains issue KC-MAJOR
                    # across four open PSUM groups, so each arriving DMA
                    # (one weight group, one x chunk) immediately unlocks
                    # four matmuls instead of one chain's worth — the PE
                    # trickles through the data-arrival window with ~1us
                    # of total idle instead of ~4us. DR tails come last
                    # (their x8/w18 operands arrive after the x bytes).
                    accs = []
                    for hc in range(4):
                        acc = hps.tile([P, TT], f32, tag="hps", name="acc")
                        accs.append(acc)
                    # issue order = DMA arrival order: (ch0,1 x kc0,1) on
                    # xF01+w1F[0]; (ch2,3 x kc0,1) on w1F[1]; (all x kc2)
                    # on xF2; (all x kc3-5) on xF345 — so the PE FIFO never
                    # holds a runnable matmul behind a data-blocked one.
                    for hc in range(4, 8):
                        acc = ops.tile([P, TT], f32, tag="ops", name="acc6")
                        accs.append(acc)
                    head = ([(hc, kc) for hc in (0, 1) for kc in (0, 1)] +
                            [(hc, kc) for hc in (2, 3) for kc in (0, 1)] +
                            [(hc, 2) for hc in range(4)] +
                            [(hc, kc) for hc in range(4) for kc in (3, 4, 5)] +
                            [(hc, kc) for hc in (4, 5) for kc in range(KF)] +
                            [(hc, kc) for hc in (6, 7) for kc in range(KF)])
                    for hc, kc in head:
                        q, r = divmod(hc, HPG)
                        nc.tensor.matmul(
                            accs[hc][:],
                            w1_t[q][:, kc * HQ + r * P:kc * HQ + r * P + P],
                            x_t[:, kc * TT:(kc + 1) * TT],
                            start=(kc == 0),
                            stop=False,
                        )
                    for hc in range(8):
                        nc.tensor.matmul(
                            accs[hc][:],
                            w18_t[:, :, hc * P:(hc + 1) * P],
                            x8_t[:],
                            start=False,
                            stop=True,
                            perf_mode=DR,
                        )
                        h = hsb.tile([P, TT], fp16, tag=f"h_{hc}")
                        h_t.append(h)
                        nc.scalar.activation(
                            h[:], accs[hc][:], GELU, bias=b1_t[:, hc:hc + 1],
                            scale=1.0 / (XS * WS),
                        )
                for hc in range(8 if ti == 0 else 0, HC):
                    q, r = divmod(hc, HPG)
                    acc = hps.tile([P, TT], f32, tag="hps")
                    for kc in range(KF):
                        nc.tensor.matmul(
                            acc[:],
                            w1_t[q][:, kc * HQ + r * P:kc * HQ + r * P + P],
                            x_t[:, kc * TT:(kc + 1) * TT],
                            start=(kc == 0),
                            stop=False,
                        )
                    nc.tensor.matmul(
                        acc[:],
                        w18_t[:, :, hc * P:(hc + 1) * P],
                        x8_t[:],
                        start=False,
                        stop=True,
                        perf_mode=DR,
                    )
                    if hc < HF:
                        h = hsb.tile([P, TT], fp16, tag=f"h_{hc}")
                        h_t.append(h)
                        out_ap = h[:]
                    else:
                        out_ap = h8_t[:, hc - HF, :]
                    nc.scalar.activation(
                        out_ap, acc[:], GELU, bias=b1_t[:, hc:hc + 1],
                        scale=1.0 / (XS * WS),
                    )

                # fc2 (shared) + expert projection: out^T[out, tok]. The
                # very last chain of the kernel runs as two half-token
                # chains so its first eviction+store overlaps the second
                # half's matmuls, shortening the serial tail (a finer
                # quarter split measured ~1.5us WORSE).
                for oc in range(OC):
                    last = (ti == NTILES - 1) and (oc == OC - 1)
                    for t1, tw in ([(0, TT // 2), (TT // 2, TT // 2)] if last
                                   else [(0, TT)]):
                        acc = ops.tile([P, TT], f32, tag="ops")
                        for hc in range(HF):
                            if oc < SC:
                                g, j = divmod(hc, W2G)
                                w = w2_t[g][:, j * SHARED + oc * P:j * SHARED + (oc + 1) * P]
                            else:
                                w = we_cur[:, hc * PART + (oc - SC) * P:hc * PART + (oc - SC + 1) * P]
                            nc.tensor.matmul(
                                acc[:, 0:tw], w, h_t[hc][:, t1:t1 + tw],
                                start=(hc == 0), stop=False,
                            )
                        w8 = (w28_t[:, :, oc * P:(oc + 1) * P] if oc < SC
                              else we8_cur[:, :, (oc - SC) * P:(oc - SC + 1) * P])
                        nc.tensor.matmul(
                            acc[:, 0:tw], w8, h8_t[:, :, t1:t1 + tw],
                            start=False, stop=True, perf_mode=DR,
                        )
                        o = osb.tile([P, TT], fp16, tag="o")
                        nc.scalar.activation(
                            o[:, 0:tw], acc[:, 0:tw], IDENT,
                            bias=b2_t[:, b * OC + oc:b * OC + oc + 1],
                            scale=1.0 / W2S,
                        )
                        nc.sync.dma_start(
                            outT_d[oc * P:(oc + 1) * P, t0 + t1:t0 + t1 + tw],
                            o[:, 0:tw],
                        )

    nc.finalize()
    return nc


def _get_program():
    if "nc" not in _CACHE:
        _CACHE["nc"] = _build_program()
    return _CACHE["nc"]


def _to_e4m3(a):
    return np.clip(a, -240, 240).astype(ml_dtypes.float8_e4m3)


def _prep_in_maps(x, indices, fc1_w, fc1_b, fc2_w, fc2_b, experts_w, experts_b):
    fp16 = np.float16
    x = np.asarray(x, dtype=np.float32)
    indices = np.asarray(indices).astype(np.int64)
    fc1_w = np.asarray(fc1_w, dtype=np.float32)
    fc1_b = np.asarray(fc1_b, dtype=np.float32)
    fc2_w = np.asarray(fc2_w, dtype=np.float32)
    fc2_b = np.asarray(fc2_b, dtype=np.float32)
    experts_w = np.asarray(experts_w, dtype=np.float32)
    experts_b = np.asarray(experts_b, dtype=np.float32)

    HQ = HID // W1Q
    H8 = HID // W18G
    # w1T = WS * fc1_w.T : [DIM, HID]; chunks kc<KF fp16, kc 6,7 fp8
    w1T = fc1_w.T * WS                                    # [DIM, HID]
    w1F = np.ascontiguousarray(
        w1T[:KF * P].reshape(KF, P, W1Q, HQ).transpose(2, 1, 0, 3)
    ).astype(fp16).reshape(W1Q, P, KF * HQ)
    # w18[g, p, s, c] = w1T[(KF+s)*P + p, g*H8 + c]
    w18 = _to_e4m3(np.ascontiguousarray(
        w1T[KF * P:].reshape(2, P, W18G, H8).transpose(2, 1, 0, 3)
    )).reshape(W18G, P, 2 * H8)
    b1T = np.ascontiguousarray(fc1_b.reshape(HC, P).T)    # [P, HC]
    # w2T = W2S * fc2_w.T : [HID, SHARED]; chunks hc<HF fp16, hc 30,31 fp8
    w2T = fc2_w.T * W2S                                   # [HID, SHARED]
    # w2P[g, p, j, s] = w2T[(g*W2G+j)*P+p, s]
    w2P = np.ascontiguousarray(
        w2T[:HF * P].reshape(HF // W2G, W2G, P, SHARED).transpose(0, 2, 1, 3)
    ).astype(fp16).reshape(HF // W2G, P, W2G * SHARED)
    # w28[p, sl, s] = w2T[(HF+sl)*P+p, s]
    w28 = _to_e4m3(np.ascontiguousarray(
        w2T[HF * P:].reshape(2, P, SHARED).transpose(1, 0, 2)
    )).reshape(P, 2 * SHARED)

    in_maps = []
    for c in range(NCORES):
        idx = indices[c * BPC:(c + 1) * BPC]              # [BPC]
        xs = x[c * BPC:(c + 1) * BPC]                     # [BPC, N, DIM]
        xT = xs.reshape(TOK, DIM).T * XS                  # [DIM, TOK]
        # xF[ti, p, kc, t] = xT[kc*P+p, ti*TT+t] for kc<KF ; fp16
        xF = np.ascontiguousarray(
            xT[:KF * P].reshape(KF, P, NTILES, TT).transpose(2, 1, 0, 3)
        ).astype(fp16).reshape(NTILES, P, KF * TT)
        # x8[ti, p, s, t] = xT[(KF+s)*P+p, ti*TT+t] ; fp8
        x8 = _to_e4m3(np.ascontiguousarray(
            xT[KF * P:].reshape(2, P, NTILES, TT).transpose(2, 1, 0, 3)
        )).reshape(NTILES, P, 2 * TT)
        # weP[b, p, hc, s] = W2S * experts_w[idx[b]].T[hc*P+p, s] ; rows 15KB
        weT = experts_w[idx].transpose(0, 2, 1) * W2S     # [BPC, HID, PART]
        weP = np.ascontiguousarray(
            weT[:, :HF * P].reshape(BPC, HF, P, PART).transpose(0, 2, 1, 3)
        ).astype(fp16).reshape(BPC, P, HF * PART)
        # we8[b, p, sl, s] = W2S * weT[b, (HF+sl)*P+p, s]
        we8 = _to_e4m3(np.ascontiguousarray(
            weT[:, HF * P:].reshape(BPC, 2, P, PART).transpose(0, 2, 1, 3)
        )).reshape(BPC, P, 2 * PART)
        b2 = np.concatenate(
            [np.broadcast_to(fc2_b, (BPC, SHARED)), experts_b[idx]], axis=1
        )                                                 # [BPC, OUT]
        b2T = np.ascontiguousarray(
            b2.reshape(BPC, OC, P).transpose(2, 0, 1).reshape(P, BPC * OC)
        ).astype(np.float32)                              # [P, BPC*OC]
        in_maps.append({
            "xF": xF, "x8": x8, "w1F": w1F, "w18": w18, "b1T": b1T,
            "w2P": w2P, "w28": w28, "weP": weP, "we8": we8, "b2T": b2T,
        })
    return in_maps


def _assemble_output(results):
    out = np.empty((B, N, OUT), dtype=np.float32)
    for c in range(NCORES):
        outT = results[c]["outT"]                         # [OUT, TOK] fp16
        out[c * BPC:(c + 1) * BPC] = (
            outT.astype(np.float32).T.reshape(BPC, N, OUT)
        )
    return out


def run_on_device(inputs: dict, trace: bool = False):
    """Run the SPMD program; returns (full_output, BassKernelResults)."""
    from concourse.bass_utils import run_bass_kernel_spmd

    nc = _get_program()
    in_maps = _prep_in_maps(**inputs)
    res = run_bass_kernel_spmd(nc, in_maps, list(range(NCORES)), trace=trace)
    return _assemble_output(res.results), res


def kernel(**inputs) -> np.ndarray:
    out, _ = run_on_device(inputs, trace=False)
    return out


# revision 56
# speedup vs baseline: 1.0344x; 1.0344x over previous
"""Trainium2 Bass kernel for MoE feed-forward (nn_MoEFeedForward_12292196401617).

Reference computation (per batch b of 32, N=1024 tokens, DIM=1024):
    h      = gelu_erf(x @ fc1_w.T + fc1_b)                  # [B,N,HID=4096]
    shared = h @ fc2_w.T + fc2_b                            # [B,N,768]
    expert = h @ experts_w[idx[b]].T + experts_b[idx[b]]    # [B,N,256]
    out    = concat([shared, expert], -1)                   # [B,N,1024]

Strategy: data-parallel over batch across 8 NeuronCores (4 batches/core).
The expert gather is resolved on the host (indices are per-batch), so the
device program is pure dense matmul, feature-on-partitions / tokens-on-free,
host-packed so every DMA row is a multi-KB contiguous DRAM run.

Precision: the kernel is PE-streaming-bound at the fp16 rate (1 row/cycle,
~874us/core), so the last 2 of 8 fc1 contraction chunks AND the last 2 of
32 fc2 chunks each run as a single fp8-e4m3 DoubleRow matmul (2 rows/
cycle, rhs free=1024): ~209ns saved per chain (DR matmul measured ~223ns
vs 432ns for 2 fp16 MMs, its 136ns LDWEIGHTS hides under the previous MM).
Scale matching lets each fp8 MM accumulate into the same PSUM group as the
fp16 MMs: operands are pre-scaled by powers of 2 in BOTH dtypes (exact in
fp16) — x*16 and fc1_w*512 (all products carry 8192, gelu eviction scale
1/8192), fc2_w/experts_w*1024 with h unscaled (products carry 1024,
identity eviction scale 1/1024; h chunks 30,31 are evicted by the gelu
straight to fp8). Quantization error scales as sqrt(fraction quantized):
full-fp8 fails the 2e-2 gate at ~5e-2, this split measures 1.8880e-2 —
bit-stable across runs and digit-identical to the numpy simulation, so the
margin is deterministic, not statistical. fp16-only measured 3.8e-4.
Output is stored fp16 and upcast on the host. Measured ~839us (baseline
fp16 kernel: 907us): ~824us matmul span at ~98% PE busy + ~8us NEFF
preamble + ~3.6us warmup bridge + ~4us ramp holes + ~4us epilogue.
"""

import sys

sys.path.insert(0, "/opt/trn_rl_repo")

import numpy as np
import ml_dtypes

B, N, DIM = 32, 1024, 1024
HID = 4096
PART = 256
OUT = 1024
SHARED = OUT - PART  # 768
E = 16

NCORES = 8
BPC = B // NCORES        # batches per core = 4
TOK = BPC * N            # tokens per core  = 4096
TT = 512                 # token tile
NTILES = TOK // TT       # 8  (2 tiles per batch)
P = 128
KC = DIM // P            # 8  fc1 contraction chunks
KF = 6                   # fc1 chunks 0..5 in fp16; 6,7 in one fp8 DoubleRow
HC = HID // P            # 32 hidden chunks
OC = OUT // P            # 8  output chunks (6 shared + 2 expert)
SC = SHARED // P         # 6
W1Q = 16                 # w1 column-slice groups (HC/W1Q = 2 hid chunks each)
W18G = 4                 # fp8 w1 DMA split along hid (2KB rows; finer
                         # splits lower packet-rate-limited early DMA BW)
HF = 30                  # fc2 chunks 0..29 in fp16; 30,31 in one fp8 DoubleRow
W2G = 6                  # fp16 w2 hid chunks packed per DMA (5 groups of 6)
XS = 16.0                # x pre-scale  (both dtypes)
WS = 512.0               # w1 pre-scale (both dtypes)
W2S = 1024.0             # w2/experts pre-scale (both dtypes)

_CACHE: dict = {}


def _build_program():
    import concourse.tile as tile
    from concourse import bacc, mybir

    fp16 = mybir.dt.float16
    fp8 = mybir.dt.float8e4
    f32 = mybir.dt.float32
    GELU = mybir.ActivationFunctionType.Gelu
    IDENT = mybir.ActivationFunctionType.Identity
    DR = mybir.MatmulPerfMode.DoubleRow

    HQ = HID // W1Q          # 256 hid cols per w1 fp16 slice group
    H8 = HID // W18G         # 1024 hid cols per w1 fp8 DMA group
    nc = bacc.Bacc()
    # packed layouts: [.., P, ..] second-to-last dim is the SBUF partition,
    # the trailing dims are one contiguous row per partition.
    xF_d = nc.declare_dram_parameter("xF", [NTILES, P, KF * TT], fp16, isOutput=False)
    x8_d = nc.declare_dram_parameter("x8", [NTILES, P, 2 * TT], fp8, isOutput=False)
    w1F_d = nc.declare_dram_parameter("w1F", [W1Q, P, KF * HQ], fp16, isOutput=False)
    w18_d = nc.declare_dram_parameter("w18", [W18G, P, 2 * H8], fp8, isOutput=False)
    b1T_d = nc.declare_dram_parameter("b1T", [P, HC], f32, isOutput=False)
    w2P_d = nc.declare_dram_parameter("w2P", [HF // W2G, P, W2G * SHARED], fp16, isOutput=False)
    w28_d = nc.declare_dram_parameter("w28", [P, 2 * SHARED], fp8, isOutput=False)
    weP_d = nc.declare_dram_parameter("weP", [BPC, P, HF * PART], fp16, isOutput=False)
    we8_d = nc.declare_dram_parameter("we8", [BPC, P, 2 * PART], fp8, isOutput=False)
    b2T_d = nc.declare_dram_parameter("b2T", [P, BPC * OC], f32, isOutput=False)
    outT_d = nc.declare_dram_parameter("outT", [OUT, TOK], fp16, isOutput=True)

    with tile.TileContext(nc) as tc:
        with (
            tc.tile_pool(name="wsb", bufs=1) as wsb,      # resident weights
            tc.tile_pool(name="wesb", bufs=1) as wesb,    # expert weights (per batch)
            tc.tile_pool(name="bsb", bufs=1) as bsb,      # biases
            tc.tile_pool(name="xsb", bufs=2) as xsb,      # x fp16 tiles, double buffered
            tc.tile_pool(name="x8sb", bufs=2) as x8sb,    # x fp8 tiles
            tc.tile_pool(name="hsb", bufs=1) as hsb,      # gelu output chunks
            tc.tile_pool(name="osb", bufs=4) as osb,      # out staging
            tc.tile_pool(name="hps", bufs=4, space="PSUM") as hps,
            tc.tile_pool(name="ops", bufs=4, space="PSUM") as ops,
        ):
            # ---- load order matters: the first fc1 chain needs x tile 0,
            # w1F group 0, x8 tile 0 and w18 group 0 within ~1.5us of compute
            # start; w2 / expert weights aren't read until the PE is ~50us
            # in, so they load behind the critical path.
            # b1/b2 are tiny (16KB each) but must NOT ride the gpsimd
            # software-DMA queue: its start time is erratic (10.6-15.5us
            # observed) while the first gelu eviction reads b1 at ~16us —
            # a straggle would stall every fc1 eviction. They ride the
            # sync ring just behind the first-chain prefix instead.
            b1_t = bsb.tile([P, HC], f32, tag="b1")
            b2_t = bsb.tile([P, BPC * OC], f32, tag="b2")

            # PE warmup: dummy matmuls on a memset scratch tile keep the PE
            # busy from the preamble until the first x/w1 bytes land, so the
            # HAM clock gate reaches 8/8 before real work starts. The
            # results are never read.
            scr = bsb.tile([P, TT], fp16, tag="scr")
            nc.vector.memset(scr[:], 0.0)
            # Engine queues are barrier-held until ~8us (NEFF preamble), so
            # the bridge covers ~8.0us -> first-chain data arrival (~10.5us
            # at the ~250-270GB/s early DMA rate; the qSync ring start
            # jitters run-to-run by ~1.5us). 18 dummies = ~3.8us of
            # continuous PE activity: reliably spans one full 3.41us HAM
            # SHORT window so the clock is at 8/8 before the first real
            # chain (15 dummies = 3.2us measured a cold ramp: every early
            # matmul at 427ns instead of 216ns). The qAct ring initializes
            # ~1.4-3.4us later than qSync, so critical loads must NOT ride
            # it (measured: moving w1F[0]/x8/w18[0] there cost 1.6-4us).
            for _ in range(18):
                wp = hps.tile([P, TT], f32, tag="hps", name="warm")
                nc.tensor.matmul(
                    wp[:, 0:256], scr[:, 0:P], scr[:, 0:256], start=True, stop=True
                )

            def load_x(ti):
                # three DMAs per token tile: the fc1 chains' kc=0..2 matmuls
                # only depend on the first fp16 half, so the PE can start
                # before the full tile lands.
                t = xsb.tile([P, KF * TT], fp16, tag="xt", name="xt")
                half = KF * TT // 2
                nc.sync.dma_start(t[:, 0:half], xF_d[ti, :, 0:half])
                t8 = x8sb.tile([P, 2, TT], fp8, tag="x8", name="x8")
                nc.sync.dma_start(t8[:], x8_d[ti])
                nc.sync.dma_start(t[:, half:], xF_d[ti, :, half:])
                return t, t8

            def load_we(b):
                # two DMAs per batch: fp16 [P, HF*PART] (15KB rows) + the
                # fp8 DoubleRow tail [P, 2, PART]
                t = wesb.tile([P, HF * PART], fp16, tag="we", name="we")
                nc.sync.dma_start(t[:], weP_d[b])
                t8 = wesb.tile([P, 2, PART], fp8, tag="we8", name="we8")
                nc.sync.dma_start(t8[:], we8_d[b])
                return t, t8

            # Tile 0 is loaded inline, split fine-grained and ordered by
            # when each byte is first needed: chain 0's kc0,1 matmuls are
            # runnable after a 450KB prefix (xF kc0-1 + w1F[0] kc0-2 half),
            # so even when the DMA ring starts late the PE gets real work
            # in a trickle instead of one long idle that re-throttles HAM.
            xt0 = xsb.tile([P, KF * TT], fp16, tag="xt", name="xt")
            nc.sync.dma_start(xt0[:, 0:2 * TT], xF_d[0, :, 0:2 * TT])
            w1_t = []
            w18_t = wsb.tile([P, 2, HID], fp8, tag="w18", name="w18")
            t = wsb.tile([P, KF * HQ], fp16, tag="w1_0", name="w1_0")
            nc.sync.dma_start(t[:], w1F_d[0])
            w1_t.append(t)
            # w1F[1] rides BEFORE the bulk x bytes: the ti==0 head issues
            # kc-major across four open chains, so one early weight group
            # unlocks 2x the runnable matmuls per arriving x byte.
            t = wsb.tile([P, KF * HQ], fp16, tag="w1_1", name="w1_1")
            nc.sync.dma_start(t[:], w1F_d[1])
            w1_t.append(t)
            nc.sync.dma_start(xt0[:, 2 * TT:3 * TT], xF_d[0, :, 2 * TT:3 * TT])
            nc.sync.dma_start(xt0[:, 3 * TT:], xF_d[0, :, 3 * TT:])
            # w1F[2] also precedes the DR operands: head chains 4,5 run
            # their fp16 matmuls while x8/w18 are still in flight.
            t = wsb.tile([P, KF * HQ], fp16, tag="w1_2", name="w1_2")
            nc.sync.dma_start(t[:], w1F_d[2])
            w1_t.append(t)
            x8t0 = x8sb.tile([P, 2, TT], fp8, tag="x8", name="x8")
            nc.sync.dma_start(x8t0[:], x8_d[0])
            nc.sync.dma_start(w18_t[:, :, 0:H8], w18_d[0])
            nc.sync.dma_start(b1_t[:], b1T_d[:, :])
            nc.sync.dma_start(b2_t[:], b2T_d[:, :])
            x_pend = (xt0, x8t0)

            # remaining w1 fp16: W1Q slice-group tiles [P, KF*HQ], one DMA
            # each, earliest-needed first (chain hc uses group
            # hc // (HC//W1Q)), interleaved with the fp8 tail weights.
            for q in range(3, W1Q):
                t = wsb.tile([P, KF * HQ], fp16, tag=f"w1_{q}", name=f"w1_{q}")
                nc.sync.dma_start(t[:], w1F_d[q])
                w1_t.append(t)
                if 0 < q - 2 < W18G:
                    nc.sync.dma_start(
                        w18_t[:, :, (q - 2) * H8:(q - 1) * H8], w18_d[q - 2]
                    )

            # w2: HF//W2G group tiles [P, W2G*SHARED], one DMA each (9KB
            # rows), plus the fp8 DoubleRow tail [P, 2, SHARED]
            w2_t = []
            for g in range(HF // W2G):
                t = wsb.tile([P, W2G * SHARED], fp16, tag=f"w2_{g}", name=f"w2_{g}")
                nc.sync.dma_start(t[:], w2P_d[g])
                w2_t.append(t)
            w28_t = wsb.tile([P, 2, SHARED], fp8, tag="w28", name="w28")
            nc.sync.dma_start(w28_t[:], w28_d[:, :])

            we_cur, we8_cur = load_we(0)

            HPG = HC // W1Q  # hid chunks per w1 slice group = 2
            for ti in range(NTILES):
                b = ti // (NTILES // BPC)
                t0 = ti * TT
                if ti % (NTILES // BPC) == 0 and ti > 0:
                    we_cur, we8_cur = load_we(b)

                x_t, x8_t = x_pend
                if ti + 1 < NTILES:
                    x_pend = load_x(ti + 1)

                # fc1 + erf-gelu: h^T[hid, tok] per 128-row chunk.
                # kc 0..5 are fp16 matmuls; kc 6,7 are one fp8 DoubleRow
                # matmul (slot s = chunk 6+s) accumulating into the same
                # PSUM tile (operand pre-scales make all products 8192*x*w).
                # Chunks hc<HF evict to fp16; hc 30,31 evict straight to the
                # fp8 tile feeding fc2's own DoubleRow tail.
                h_t = []
                h8_t = hsb.tile([P, 2, TT], fp8, tag="h8")
                if ti == 0:
                    # Ramp head: the first four chains issue KC-MAJOR
                    # across four open PSUM groups, so each arriving DMA
                    # (one weight group, one x chunk) immediately unlocks
                    # four matmuls instead of one chain's worth — the PE
                    # trickles through the data-arrival window with ~1us
                    # of total idle instead of ~4us. DR tails come last
                    # (their x8/w18 operands arrive after the x bytes).
                    accs = []
                    for hc in range(4):
                        acc = hps.tile([P, TT], f32, tag="hps", name="acc")
                        accs.append(acc)
                    # issue order = DMA arrival order: (ch0,1 x kc0,1) on
                    # xF01+w1F[0]; (ch2,3 x kc0,1) on w1F[1]; (all x kc2)
                    # on xF2; (all x kc3-5) on xF345 — so the PE FIFO never
                    # holds a runnable matmul behind a data-blocked one.
                    for hc in range(4, 6):
                        acc = ops.tile([P, TT], f32, tag="ops", name="acc6")
                        accs.append(acc)
                    head = ([(hc, kc) for hc in (0, 1) for kc in (0, 1)] +
                            [(hc, kc) for hc in (2, 3) for kc in (0, 1)] +
                            [(hc, 2) for hc in range(4)] +
                            [(hc, kc) for hc in range(4) for kc in (3, 4, 5)] +
                            [(hc, kc) for hc in (4, 5) for kc in range(KF)])
                    for hc, kc in head:
                        q, r = divmod(hc, HPG)
                        nc.tensor.matmul(
                            accs[hc][:],
                            w1_t[q][:, kc * HQ + r * P:kc * HQ + r * P + P],
                            x_t[:, kc * TT:(kc + 1) * TT],
                            start=(kc == 0),
                            stop=False,
                        )
                    for hc in range(6):
                        nc.tensor.matmul(
                            accs[hc][:],
                            w18_t[:, :, hc * P:(hc + 1) * P],
                            x8_t[:],
                            start=False,
                            stop=True,
                            perf_mode=DR,
                        )
                        h = hsb.tile([P, TT], fp16, tag=f"h_{hc}")
                        h_t.append(h)
                        nc.scalar.activation(
                            h[:], accs[hc][:], GELU, bias=b1_t[:, hc:hc + 1],
                            scale=1.0 / (XS * WS),
                        )
                for hc in range(6 if ti == 0 else 0, HC):
                    q, r = divmod(hc, HPG)
                    acc = hps.tile([P, TT], f32, tag="hps")
                    for kc in range(KF):
                        nc.tensor.matmul(
                            acc[:],
                            w1_t[q][:, kc * HQ + r * P:kc * HQ + r * P + P],
                            x_t[:, kc * TT:(kc + 1) * TT],
                            start=(kc == 0),
                            stop=False,
                        )
                    nc.tensor.matmul(
                        acc[:],
                        w18_t[:, :, hc * P:(hc + 1) * P],
                        x8_t[:],
                        start=False,
                        stop=True,
                        perf_mode=DR,
                    )
                    if hc < HF:
                        h = hsb.tile([P, TT], fp16, tag=f"h_{hc}")
                        h_t.append(h)
                        out_ap = h[:]
                    else:
                        out_ap = h8_t[:, hc - HF, :]
                    nc.scalar.activation(
                        out_ap, acc[:], GELU, bias=b1_t[:, hc:hc + 1],
                        scale=1.0 / (XS * WS),
                    )

                # fc2 (shared) + expert projection: out^T[out, tok]. The
                # very last chain of the kernel runs as two half-token
                # chains so its first eviction+store overlaps the second
                # half's matmuls, shortening the serial tail (a finer
                # quarter split measured ~1.5us WORSE).
                for oc in range(OC):
                    last = (ti == NTILES - 1) and (oc == OC - 1)
                    for t1, tw in ([(0, TT // 2), (TT // 2, TT // 2)] if last
                                   else [(0, TT)]):
                        acc = ops.tile([P, TT], f32, tag="ops")
                        for hc in range(HF):
                            if oc < SC:
                                g, j = divmod(hc, W2G)
                                w = w2_t[g][:, j * SHARED + oc * P:j * SHARED + (oc + 1) * P]
                            else:
                                w = we_cur[:, hc * PART + (oc - SC) * P:hc * PART + (oc - SC + 1) * P]
                            nc.tensor.matmul(
                                acc[:, 0:tw], w, h_t[hc][:, t1:t1 + tw],
                                start=(hc == 0), stop=False,
                            )
                        w8 = (w28_t[:, :, oc * P:(oc + 1) * P] if oc < SC
                              else we8_cur[:, :, (oc - SC) * P:(oc - SC + 1) * P])
                        nc.tensor.matmul(
                            acc[:, 0:tw], w8, h8_t[:, :, t1:t1 + tw],
                            start=False, stop=True, perf_mode=DR,
                        )
                        o = osb.tile([P, TT], fp16, tag="o")
                        nc.scalar.activation(
                            o[:, 0:tw], acc[:, 0:tw], IDENT,
                            bias=b2_t[:, b * OC + oc:b * OC + oc + 1],
                            scale=1.0 / W2S,
                        )
                        nc.sync.dma_start(
                            outT_d[oc * P:(oc + 1) * P, t0 + t1:t0 + t1 + tw],
                            o[:, 0:tw],
                        )

    nc.finalize()
    return nc


def _get_program():
    if "nc" not in _CACHE:
        _CACHE["nc"] = _build_program()
    return _CACHE["nc"]


def _to_e4m3(a):
    return np.clip(a, -240, 240).astype(ml_dtypes.float8_e4m3)


def _prep_in_maps(x, indices, fc1_w, fc1_b, fc2_w, fc2_b, experts_w, experts_b):
    fp16 = np.float16
    x = np.asarray(x, dtype=np.float32)
    indices = np.asarray(indices).astype(np.int64)
    fc1_w = np.asarray(fc1_w, dtype=np.float32)
    fc1_b = np.asarray(fc1_b, dtype=np.float32)
    fc2_w = np.asarray(fc2_w, dtype=np.float32)
    fc2_b = np.asarray(fc2_b, dtype=np.float32)
    experts_w = np.asarray(experts_w, dtype=np.float32)
    experts_b = np.asarray(experts_b, dtype=np.float32)

    HQ = HID // W1Q
    H8 = HID // W18G
    # w1T = WS * fc1_w.T : [DIM, HID]; chunks kc<KF fp16, kc 6,7 fp8
    w1T = fc1_w.T * WS                                    # [DIM, HID]
    w1F = np.ascontiguousarray(
        w1T[:KF * P].reshape(KF, P, W1Q, HQ).transpose(2, 1, 0, 3)
    ).astype(fp16).reshape(W1Q, P, KF * HQ)
    # w18[g, p, s, c] = w1T[(KF+s)*P + p, g*H8 + c]
    w18 = _to_e4m3(np.ascontiguousarray(
        w1T[KF * P:].reshape(2, P, W18G, H8).transpose(2, 1, 0, 3)
    )).reshape(W18G, P, 2 * H8)
    b1T = np.ascontiguousarray(fc1_b.reshape(HC, P).T)    # [P, HC]
    # w2T = W2S * fc2_w.T : [HID, SHARED]; chunks hc<HF fp16, hc 30,31 fp8
    w2T = fc2_w.T * W2S                                   # [HID, SHARED]
    # w2P[g, p, j, s] = w2T[(g*W2G+j)*P+p, s]
    w2P = np.ascontiguousarray(
        w2T[:HF * P].reshape(HF // W2G, W2G, P, SHARED).transpose(0, 2, 1, 3)
    ).astype(fp16).reshape(HF // W2G, P, W2G * SHARED)
    # w28[p, sl, s] = w2T[(HF+sl)*P+p, s]
    w28 = _to_e4m3(np.ascontiguousarray(
        w2T[HF * P:].reshape(2, P, SHARED).transpose(1, 0, 2)
    )).reshape(P, 2 * SHARED)

    in_maps = []
    for c in range(NCORES):
        idx = indices[c * BPC:(c + 1) * BPC]              # [BPC]
        xs = x[c * BPC:(c + 1) * BPC]                     # [BPC, N, DIM]
        xT = xs.reshape(TOK, DIM).T * XS                  # [DIM, TOK]
        # xF[ti, p, kc, t] = xT[kc*P+p, ti*TT+t] for kc<KF ; fp16
        xF = np.ascontiguousarray(
            xT[:KF * P].reshape(KF, P, NTILES, TT).transpose(2, 1, 0, 3)
        ).astype(fp16).reshape(NTILES, P, KF * TT)
        # x8[ti, p, s, t] = xT[(KF+s)*P+p, ti*TT+t] ; fp8
        x8 = _to_e4m3(np.ascontiguousarray(
            xT[KF * P:].reshape(2, P, NTILES, TT).transpose(2, 1, 0, 3)
        )).reshape(NTILES, P, 2 * TT)
        # weP[b, p, hc, s] = W2S * experts_w[idx[b]].T[hc*P+p, s] ; rows 15KB
        weT = experts_w[idx].transpose(0, 2, 1) * W2S     # [BPC, HID, PART]
        weP = np.ascontiguousarray(
            weT[:, :HF * P].reshape(BPC, HF, P, PART).transpose(0, 2, 1, 3)
        ).astype(fp16).reshape(BPC, P, HF * PART)
        # we8[b, p, sl, s] = W2S * weT[b, (HF+sl)*P+p, s]
        we8 = _to_e4m3(np.ascontiguousarray(
            weT[:, HF * P:].reshape(BPC, 2, P, PART).transpose(0, 2, 1, 3)
        )).reshape(BPC, P, 2 * PART)
        b2 = np.concatenate(
            [np.broadcast_to(fc2_b, (BPC, SHARED)), experts_b[idx]], axis=1
        )                                                 # [BPC, OUT]
        b2T = np.ascontiguousarray(
            b2.reshape(BPC, OC, P).transpose(2, 0, 1).reshape(P, BPC * OC)
        ).astype(np.float32)                              # [P, BPC*OC]
        in_maps.append({
            "xF": xF, "x8": x8, "w1F": w1F, "w18": w18, "b1T": b1T,
            "w2P": w2P, "w28": w28, "weP": weP, "we8": we8, "b2T": b2T,
        })
    return in_maps


def _assemble_output(results):
    out = np.empty((B, N, OUT), dtype=np.float32)
    for c in range(NCORES):
        outT = results[c]["outT"]                         # [OUT, TOK] fp16
        out[c * BPC:(c + 1) * BPC] = (
            outT.astype(np.float32).T.reshape(BPC, N, OUT)
        )
    return out


def run_on_device(inputs: dict, trace: bool = False):
    """Run the SPMD program; returns (full_output, BassKernelResults)."""
    from concourse.bass_utils import run_bass_kernel_spmd

    nc = _get_program()
    in_maps = _prep_in_maps(**inputs)
    res = run_bass_kernel_spmd(nc, in_maps, list(range(NCORES)), trace=trace)
    return _assemble_output(res.results), res


def kernel(**inputs) -> np.ndarray:
    out, _ = run_on_device(inputs, trace=False)
    return out
